# revision 2
# baseline (speedup 1.0000x reference)
"""CoPEGate Trainium2 kernel (v2).

Computes out[b,h,t,s] = sigmoid((Q K^T)[b,h,t,s] / sqrt(D)) * (P P^T)[t,s] / sqrt(D)
for B=2, H=12, T=2048, D=64 (fp32 in/out), distributed over 8 NeuronCores.

Sharding: the 24 (b,h) pairs are split 3-per-core (head-parallel). The
positional bias P P^T / sqrt(D) is computed ON THE HOST (a single
2048x2048x64 GEMM, i.e. input preprocessing of the replicated small
operand per the sharding hint), cast to fp16 in SBUF layout, and shipped
as a replicated input. No cross-device communication.

Why v2 beats v1 (119 us, trace-measured):

1. v1's pacer was ACT: 96 sigmoids of [128,1024] at 997 ns start-to-
   start (= (1024+172)/1.2GHz, exact) = 100.7 us. 2048-wide sigmoids
   run at (2048+172)/1.2 = 1850 ns -> 48 x 1850 = 88.8 us. v1 could
   not go 2048-wide: with the pos bias transiting PSUM, per-tile PSUM
   bank-tenancy (writes + reads) exactly equals the available 2-slot
   time, so wide stripes stall structurally.

2. Shipping pos from the host removes the PE pos matmuls, the 39 us of
   DVE PSUM->SBUF f32->f16 casts (DVE drops to ~59 us of muls, well
   under ACT), and all pos PSUM tenancy -- so gates get all 8 PSUM
   banks as a 2 x [128,2048] ping-pong and ACT streams back-to-back.

3. DMA budget: out 24 MiB + pos 8 MiB + q/k 1.75 MiB = 33.75 MiB vs
   ~358 GB/s/core. pos chunks are size-ramped (0.5/0.5/1/2/4 MiB) and
   issued early so tile j's slice always lands before its muls; the
   ~7 us framework preamble and ramp give the prefetch a head start.

Steady state per row-tile (16 tiles):
  PE : 12 x [128(K),512] fp16 matmul chunks (~4.3 us incl LDWEIGHTS)
  ACT: 3 x [128,2048] sigmoid PSUM->SBUF f16 (1850 ns each; pacer)
  DVE: 3 x [128,2048] fp16 tensor_mul (1226 ns each)
  DMA: 3 x 512 KiB output stripes (+ pos trickle)
Precision: q/k fp16 (f32 psum accumulate), pos f32 host GEMM -> fp16,
fp16 out upcast on host; rel err ~5e-4 vs the 2e-2 gate.
"""

import math
import os
import sys

import numpy as np

sys.path.insert(0, "/opt/trn_rl_repo")

B, H, T, D = 2, 12, 2048, 64
N_CORES = 8
HPC = (B * H) // N_CORES  # heads per core
PT = 128  # output row-tile height (SBUF/PSUM partitions)
NT = T // PT  # row tiles
NCHUNK = 512  # matmul moving-operand free dim (one PSUM bank of fp32)
NCH = T // NCHUNK
INV_SQRT_D = 1.0 / math.sqrt(D)

# pos prefetch chunking: tiles per DMA, size-ramped so early tiles land
# before their muls while later bulk amortizes issue cost.
POS_CHUNKS = (1, 1, 2, 4, 8)

_NC_CACHE = {}


def _build_nc():
    import concourse.bass as bass
    from concourse import bacc, mybir, tile

    f32 = mybir.dt.float32
    f16 = mybir.dt.float16
    Sigmoid = mybir.ActivationFunctionType.Sigmoid

    nc = bacc.Bacc("TRN2", target_bir_lowering=False)

    # Host-packed operands:
    #   QZ[h] = q_h^T [64, 2048]; the other 64 rows of each [128, T]
    #   stationary tile are memset to 0 on-device (zero rows contribute
    #   exactly 0 to the K=128 contraction, which runs the PE at 2.4GHz
    #   vs 1.2 for K=64).
    #   RHS[0] = [k0;k1], RHS[1] = [k2;k2] (moving tiles, rows = K).
    #   POS[p, it*T + c] = pos_bias[it*128 + p, c] * inv_sqrt_d (fp16) --
    #   i.e. already in SBUF [partition, tile-major free] layout.
    QZ = nc.dram_tensor("QZ", [HPC, D, T], f16, kind="ExternalInput")
    RHS = nc.dram_tensor("RHS", [2, 2 * D, T], f16, kind="ExternalInput")
    POS = nc.dram_tensor("POS", [PT, NT * T], f16, kind="ExternalInput")
    out = nc.dram_tensor("out", [HPC, T, T], f16, kind="ExternalOutput")

    with tile.TileContext(nc) as tc:
        with tc.tile_pool(name="ins", bufs=1) as ins_pool, \
             tc.tile_pool(name="gate", bufs=4) as gate_pool, \
             tc.tile_pool(name="outs", bufs=8) as outs_pool, \
             tc.tile_pool(name="ps", bufs=2, space="PSUM") as ps_pool:

            qz0 = ins_pool.tile([2 * D, T], f16, tag="qz0")
            qz1 = ins_pool.tile([2 * D, T], f16, tag="qz1")
            qz2 = ins_pool.tile([2 * D, T], f16, tag="qz2")
            rk = ins_pool.tile([2 * D, T], f16, tag="rk")
            rp = ins_pool.tile([2 * D, T], f16, tag="rp")
            pos = ins_pool.tile([PT, NT * T], f16, tag="pos")

            # Zero halves: qz0=[q0;0], qz1=[0;q1], qz2=[q2;0].
            # GPSIMD + DVE are idle through the ramp; keep zeros off the
            # DMA wire.
            nc.gpsimd.memset(qz0[D : 2 * D, :], 0.0)
            nc.vector.memset(qz1[0:D, :], 0.0)
            nc.gpsimd.memset(qz2[D : 2 * D, :], 0.0)

            # Input DMAs in first-use order. rk is split into 4 chunks
            # so tile-0 matmuls start as soon as the first cols land.
            nc.sync.dma_start(out=qz0[0:D, 0:PT], in_=QZ[0][:, 0:PT])
            for j in range(NCH):
                nc.sync.dma_start(
                    out=rk[:, bass.ts(j, NCHUNK)], in_=RHS[0][:, bass.ts(j, NCHUNK)]
                )
            nc.sync.dma_start(out=qz0[0:D, PT:T], in_=QZ[0][:, PT:T])
            # pos tile 0 early: mul(0,h0) needs it ~3 us after ACT#0.
            pos_off = 0
            pos_slices = []
            for ntile in POS_CHUNKS:
                pos_slices.append((pos_off, ntile))
                pos_off += ntile
            nc.sync.dma_start(
                out=pos[:, 0:T], in_=POS[:, 0:T]
            )
            nc.sync.dma_start(out=qz1[D : 2 * D, :], in_=QZ[1][:, :])
            nc.sync.dma_start(out=pos[:, T : 2 * T], in_=POS[:, T : 2 * T])
            nc.sync.dma_start(out=rp, in_=RHS[1][:, :])
            nc.sync.dma_start(out=qz2[0:D, :], in_=QZ[2][:, :])
            for off, ntile in pos_slices[2:]:
                nc.sync.dma_start(
                    out=pos[:, off * T : (off + ntile) * T],
                    in_=POS[:, off * T : (off + ntile) * T],
                )

            lhs_t = {0: qz0, 1: qz1, 2: qz2}
            rhs_t = {0: rk, 1: rk, 2: rp}

            for it in range(NT):
                last = it == NT - 1
                for h in range(HPC):
                    ps = ps_pool.tile([PT, T], f32, tag="ps")
                    lhsT = lhs_t[h][:, bass.ts(it, PT)]
                    for j in range(NCH):
                        nc.tensor.matmul(
                            ps[:, bass.ts(j, NCHUNK)],
                            lhsT,
                            rhs_t[h][:, bass.ts(j, NCHUNK)],
                            start=True,
                            stop=True,
                        )
                    gate = gate_pool.tile([PT, T], f16, tag="gate")
                    nc.scalar.activation(gate, ps, Sigmoid, scale=INV_SQRT_D)
                    o = outs_pool.tile([PT, T], f16, tag="o")
                    pslice = pos[:, it * T : (it + 1) * T]
                    if last:
                        # Tail trim: half-width muls + DMAs so the final
                        # bytes trail the last sigmoid minimally.
                        for half in range(2):
                            hsl = bass.ts(half, T // 2)
                            nc.vector.tensor_mul(
                                o[:, hsl], gate[:, hsl], pslice[:, hsl]
                            )
                            nc.sync.dma_start(
                                out=out[h, bass.ts(it, PT), hsl], in_=o[:, hsl]
                            )
                    else:
                        nc.vector.tensor_mul(o, gate, pslice)
                        nc.sync.dma_start(out=out[h, bass.ts(it, PT), :], in_=o)

    nc.finalize()
    return nc


def _get_nc():
    if "nc" not in _NC_CACHE:
        _NC_CACHE["nc"] = _build_nc()
    return _NC_CACHE["nc"]


def kernel(query, key, pos_embed_weight):
    query = np.asarray(query, dtype=np.float32)
    key = np.asarray(key, dtype=np.float32)
    pos_embed_weight = np.asarray(pos_embed_weight, dtype=np.float32)

    q = query.reshape(B * H, T, D)
    k = key.reshape(B * H, T, D)

    # Replicated positional bias, computed on host (small GEMM over the
    # replicated operand) in f32, pre-scaled, then cast once to fp16 in
    # the exact SBUF [partition, tile-major] layout the kernel reads.
    p = pos_embed_weight[:T]
    pos_bias = (p @ p.T) * np.float32(INV_SQRT_D)
    posh = (
        pos_bias.astype(np.float16)
        .reshape(NT, PT, T)
        .transpose(1, 0, 2)
        .reshape(PT, NT * T)
    )
    posh = np.ascontiguousarray(posh)

    in_maps = []
    for c in range(N_CORES):
        hs = [c * HPC + i for i in range(HPC)]
        qz = np.empty((HPC, D, T), dtype=np.float16)
        for i, h in enumerate(hs):
            qz[i] = q[h].T
        kT = [k[h].T.astype(np.float16) for h in hs]
        rhs = np.empty((2, 2 * D, T), dtype=np.float16)
        rhs[0, :D] = kT[0]
        rhs[0, D:] = kT[1]
        rhs[1, :D] = kT[2]
        rhs[1, D:] = kT[2]
        in_maps.append({"QZ": qz, "RHS": rhs, "POS": posh})

    from concourse.bass_utils import run_bass_kernel_spmd

    nc = _get_nc()
    try:
        res = run_bass_kernel_spmd(
            nc,
            in_maps,
            core_ids=list(range(N_CORES)),
            trace=bool(os.environ.get("KERNEL_TRACE")),
        )
    except Exception:
        # One retry for transient runtime/compile hiccups.
        res = run_bass_kernel_spmd(
            nc, in_maps, core_ids=list(range(N_CORES)), trace=False
        )
    kernel.last_results = res

    full = np.empty((B * H, T, T), dtype=np.float32)
    for c in range(N_CORES):
        full[c * HPC : (c + 1) * HPC] = res.results[c]["out"]
    return full.reshape(B, H, T, T)


kernel.last_results = None
